# revision 9
# baseline (speedup 1.0000x reference)
"""Single-head attention (B=8, S=4096, D=1024, DK=DV=128) on 8 TRN2 NeuronCores.

Sharding: data-parallel over batch - one batch element per core, the three
Linear weights replicated. No collectives.

v3 (from the v2 ~212-221us baseline). Trace-driven changes:

1. ScalarE is the steady-state bottleneck (ACTIVATE = (N+352)/1.2 ns each,
   ~16.1us/q-block effective). v2 also put chunk-load DMA triggers on the
   scalar queue (~700ns each inside the ACT FIFO) - v3 issues all x-chunk
   loads on the sync HWDGE ring only (one ring's DMA already fans out to
   all 16 SDMA engines; quarters are sequenced on the same ring so
   projection subtile deps fire as early as before). Weights/biases go on
   gpsimd SWDGE.

2. v2 emitted project_qt(chunk 1..3) between attn_end(qb) and
   attn_begin(qb+1): 16 projection MMs + DVE bias adds sat between the
   q-blocks in the engine FIFOs, starving ScalarE ~3.6us at three q-block
   boundaries. v3 emits those projections inside the next q-block's
   attention stream (after group 2), where PE/DVE have slack.

3. Host staging is chunk-contiguous: x^T is pre-arranged per load-chunk as
   [ci][p][c*CH+s] so each chunk DMA is 128 descriptors x 4-16KB
   contiguous (v2: 256 x 2KB strided).

4. FP8_QK: query/key activations AND Wq/Wk are cast host-side to fp8-e4m3
   (TRN float8e4; values well under the 240 max). The Q/K projections run
   as fp8 DoubleRow matmuls (contraction pairs of d-chunks, 2 MACs/cycle):
   4 pair-MMs per 512-block instead of 8, halving Q/K projection PE time,
   and the Q/K input DMA bytes halve (front is HBM-bound at ~300GB/s/core).
   V path stays bf16 (the output is a near-uniform softmax average, so V
   quantization error does not average out; measured numpy-sim rel_err
   1.1e-2 for this config vs 1.7e-3 baseline, tolerance 2e-2).

Attention machinery unchanged from v2: transposed-score layout, exp on
ScalarE in [128,1024] calls, P^T @ [V | 1] PSUM accumulation with the ones
column producing the softmax denominator, fused DVE normalize.
"""

import math
import os

import numpy as np

B, S, D, DK, DV = 8, 4096, 1024, 128, 128
P = 128
SB = 512  # q-block width (attention) and projection block
CH = 1024  # load chunk (sequence cols per stage load)
CD = D // P  # 8 d-chunks
NSB = S // SB  # 8 q-blocks
NKC = S // P  # 32 key chunks
NCH = S // CH  # 4 load chunks per tensor
JPB = SB // P  # 4 q-subchunks per block
SCALE = 1.0 / math.sqrt(DK)

FP8_QK = os.environ.get("FP8_QK", "1") != "0"  # fp8-e4m3 x_q/x_k/Wq/Wk + DoubleRow projections

_cache = {}

# kept for test.py compat (unused)
XBAR_DUAL = False
SWDGE_QUEUES = 1


def _emit(tc, aps):
    from concourse import mybir

    nc = tc.nc
    bf16 = mybir.dt.bfloat16
    f32 = mybir.dt.float32

    qT, kT, vT, wq, wk, wv, bq, bk, bv, out = aps

    out_ap = out.rearrange("(nb j p) d -> nb p j d", p=P, j=JPB)

    from contextlib import ExitStack

    with ExitStack() as ctx:
        consts = ctx.enter_context(tc.tile_pool(name="consts", bufs=1))
        qkv = ctx.enter_context(tc.tile_pool(name="qkv", bufs=1))
        qtp = ctx.enter_context(tc.tile_pool(name="qt", bufs=NSB))
        stagep = ctx.enter_context(tc.tile_pool(name="stage", bufs=8))
        ptp = ctx.enter_context(tc.tile_pool(name="pt", bufs=4))
        outp = ctx.enter_context(tc.tile_pool(name="outp", bufs=2))
        smallp = ctx.enter_context(tc.tile_pool(name="small", bufs=4))
        psump = ctx.enter_context(tc.tile_pool(name="ps", bufs=2, space="PSUM"))

        # --- constants ---
        if FP8_QK:
            f8 = mybir.dt.float8e4
            wq_sb = consts.tile([P, CD // 2, 2, DK], f8)
            wk_sb = consts.tile([P, CD // 2, 2, DK], f8)
        else:
            wq_sb = consts.tile([P, CD, DK], bf16)
            wk_sb = consts.tile([P, CD, DK], bf16)
        wv_sb = consts.tile([P, CD, DV], bf16)
        # wk first on sync (needed by the first projections), rest on gpsimd
        nc.sync.dma_start(out=wk_sb, in_=wk)
        nc.gpsimd.dma_start(out=wv_sb, in_=wv)
        nc.gpsimd.dma_start(out=wq_sb, in_=wq)
        bq_sb = consts.tile([P, 1], f32)
        bk_sb = consts.tile([P, 1], f32)
        bv_sb = consts.tile([P, DV], f32)
        nc.gpsimd.dma_start(out=bq_sb, in_=bq)
        nc.gpsimd.dma_start(out=bk_sb, in_=bk)
        nc.gpsimd.dma_start(out=bv_sb, in_=bv)

        # warm the exp table set while loads stream
        warm_in = consts.tile([P, 8], f32)
        warm_out = consts.tile([P, 8], f32)
        nc.vector.memset(warm_in, 0.0)
        nc.scalar.activation(warm_out, warm_in, mybir.ActivationFunctionType.Exp)

        # persistent per-core tensors
        kt_sb = qkv.tile([P, S], bf16)  # K^T  [dk, s]
        vp_sb = qkv.tile([P, NKC, DV + 1], bf16)  # V' natural [s%128, chunk, dv+1]
        nc.vector.memset(vp_sb[:, :, DV : DV + 1], 1.0)
        qts = [qtp.tile([P, SB], bf16, tag="qt", name=f"qt{i}") for i in range(NSB)]

        def load_chunk(src_ap, ci, nm, dt, quarters=False):
            # sequential c-splits on the sync ring: full BW (one DMA fans out
            # to all 16 SDMA engines), and the projections' subtile deps on
            # the c dim fire as each split lands.
            st = stagep.tile([P, CD, CH], dt, tag="stage", name=f"st_{nm}{ci}")
            sl = src_ap[ci]
            step = CD // 4 if quarters else CD // 2
            for c0 in range(0, CD, step):
                nc.sync.dma_start(out=st[:, c0 : c0 + step, :], in_=sl[:, c0 : c0 + step, :])
            return st

        def project_x(st, ci, w_sb, b_sb, dst, h_lo, h_hi, nm):
            # dst: callable sb -> (out_ap) receiving the biased block
            for h in range(h_lo, h_hi):
                sb = ci * (CH // SB) + h
                ps = psump.tile([P, SB], f32, tag="sps", bufs=2, name=f"{nm}ps{sb}")
                if FP8_QK:
                    for c2 in range(CD // 2):
                        nc.tensor.matmul(
                            ps,
                            w_sb[:, c2, :, :],
                            st[:, 2 * c2 : 2 * c2 + 2, h * SB : (h + 1) * SB],
                            start=(c2 == 0),
                            stop=(c2 == CD // 2 - 1),
                            perf_mode=mybir.MatmulPerfMode.DoubleRow,
                        )
                else:
                    for c in range(CD):
                        nc.tensor.matmul(
                            ps,
                            w_sb[:, c, :],
                            st[:, c, h * SB : (h + 1) * SB],
                            start=(c == 0),
                            stop=(c == CD - 1),
                        )
                nc.vector.tensor_scalar_add(dst(sb), ps, b_sb)

        def project_kt(st, ci, h_lo=0, h_hi=CH // SB):
            project_x(
                st, ci, wk_sb, bk_sb,
                lambda sb: kt_sb[:, sb * SB : (sb + 1) * SB], h_lo, h_hi, "k",
            )

        def project_qt(st, ci, h_lo=0, h_hi=CH // SB):
            project_x(st, ci, wq_sb, bq_sb, lambda sb: qts[sb], h_lo, h_hi, "q")

        def project_v(st, ci, j_lo=0, j_hi=CH // P):
            # 4 j-outputs packed per [128, 512] psum tile; start=True only on
            # the first matmul per bank (whole-bank has_written clear), later
            # j's first write overwrites on cleared bits.
            for j4 in range(j_lo, j_hi, 4):
                vps = psump.tile(
                    [P, SB], f32, tag="sps", bufs=2, name=f"vps{ci}_{j4}"
                )
                for j in range(j4, j4 + 4):
                    jj = j - j4
                    for c in range(CD):
                        nc.tensor.matmul(
                            vps[:, jj * DV : (jj + 1) * DV],
                            st[:, c, j * P : (j + 1) * P],
                            wv_sb[:, c, :],
                            start=(jj == 0 and c == 0),
                            stop=(c == CD - 1),
                        )
                kk0 = ci * (CH // P) + j4
                nc.vector.tensor_copy(
                    vp_sb[:, kk0 : kk0 + 4, 0:DV],
                    vps.rearrange("p (j d) -> p j d", j=4),
                )

        # --- attention emission helpers ---
        # key chunks grouped 2 per exp call ([128, 1024] ACTIVATEs); the
        # smaller group buys a 3-deep sps rotation (3x2=6 banks) that
        # decouples the scores matmuls from exp WAR jitter.
        groups = [(g * 2, 2) for g in range(NKC // 2)]
        ngrp = len(groups)
        qb_ops = {}

        def attn_begin(qb):
            # ops tag has 4 bufs (4 PSUM banks) so TWO q-blocks can be in
            # flight; qb0+qb1 interleave through the DMA-bound load front.
            opsA = psump.tile([P, 2, DV + 1], f32, tag="ops", bufs=4, name=f"opsA{qb}")
            opsB = psump.tile([P, 2, DV + 1], f32, tag="ops", bufs=4, name=f"opsB{qb}")
            qb_ops[qb] = [opsA[:, 0, :], opsA[:, 1, :], opsB[:, 0, :], opsB[:, 1, :]]

        def attn_groups(qb, g_lo, g_hi):
            ops = qb_ops[qb]
            for gi in range(g_lo, g_hi):
                k0, gn = groups[gi]
                sps = psump.tile(
                    [P, gn * SB], f32, tag="sps", bufs=2, name=f"sps{qb}_{gi}"
                )
                for h in range(gn):
                    kk = k0 + h
                    nc.tensor.matmul(
                        sps[:, h * SB : (h + 1) * SB],
                        kt_sb[:, kk * P : (kk + 1) * P],
                        qts[qb],
                        start=True,
                        stop=True,
                    )
                pt = ptp.tile([P, gn * SB], bf16, tag="pt", name=f"pt{qb}_{gi}")
                nc.scalar.activation(
                    pt, sps, mybir.ActivationFunctionType.Exp, scale=SCALE
                )
                for h in range(gn):
                    kk = k0 + h
                    for j in range(JPB):
                        # start=True clears has_written for the WHOLE bank, so
                        # only the first matmul per bank (j=0 / j=2) may set it;
                        # the partner tile's first write lands on cleared bits
                        # and overwrites (per-element has_written semantics).
                        nc.tensor.matmul(
                            ops[j],
                            pt[:, h * SB + j * P : h * SB + (j + 1) * P],
                            vp_sb[:, kk, :],
                            start=(gi == 0 and h == 0 and j % 2 == 0),
                            stop=(gi == ngrp - 1 and h == gn - 1),
                        )

        def attn_end(qb):
            ops = qb_ops.pop(qb)
            ostage = outp.tile([P, JPB, DV], f32, tag="ostage", name=f"ostage{qb}")
            for j in range(JPB):
                recip = smallp.tile([P, 1], f32, tag="recip", name=f"recip{qb}_{j}")
                nc.vector.reciprocal(recip, ops[j][:, DV : DV + 1])
                nc.vector.scalar_tensor_tensor(
                    ostage[:, j, :],
                    ops[j][:, 0:DV],
                    recip,
                    bv_sb,
                    mybir.AluOpType.mult,
                    mybir.AluOpType.add,
                )
                if j % 2 == 1:  # store halves as they complete (shorter tail)
                    nc.sync.dma_start(
                        out=out_ap[qb][:, j - 1 : j + 1, :],
                        in_=ostage[:, j - 1 : j + 1, :],
                    )

        def wave(g_lo, g_hi):
            # qb0 and qb1 interleaved per group: both ride the load front, so
            # 32 ACTs (not 16) complete before the DMA-bound front ends.
            for g in range(g_lo, g_hi):
                attn_groups(0, g, g + 1)
                attn_groups(1, g, g + 1)

        # --- software pipeline in emission order (engines run their streams
        # FIFO, so emission order IS the per-engine execution order) ---
        kdt = mybir.dt.float8e4 if FP8_QK else bf16
        stk0 = load_chunk(kT, 0, "k", kdt, quarters=True)
        stq0 = load_chunk(qT, 0, "q", kdt, quarters=True)
        stv0 = load_chunk(vT, 0, "v", bf16, quarters=True)
        # first chunk's projections at block granularity so the first
        # attention groups fire as soon as kc 0..3 are projected
        project_kt(stk0, 0, 0, 1)  # kc 0..3
        project_qt(stq0, 0)  # qt[0], qt[1]
        project_v(stv0, 0, 0, 4)  # vp 0..3

        attn_begin(0)
        attn_begin(1)
        wave(0, 2)  # kc 0..3
        project_kt(stk0, 0, 1, 2)  # kc 4..7
        project_v(stv0, 0, 4, 8)  # vp 4..7
        stk = load_chunk(kT, 1, "k", kdt)
        stv = load_chunk(vT, 1, "v", bf16)
        wave(2, 4)  # kc 4..7
        project_kt(stk, 1)
        project_v(stv, 1)
        stk = load_chunk(kT, 2, "k", kdt)
        stv = load_chunk(vT, 2, "v", bf16)
        wave(4, 8)  # kc 8..15, needs chunk 1
        project_kt(stk, 2)
        project_v(stv, 2)
        stq1 = load_chunk(qT, 1, "q", kdt)
        stk = load_chunk(kT, 3, "k", kdt)
        stv = load_chunk(vT, 3, "v", bf16)
        wave(8, 12)  # kc 16..23, needs chunk 2
        project_kt(stk, 3)
        project_v(stv, 3)
        project_qt(stq1, 1, 0, 1)  # qts[2]
        stq2 = load_chunk(qT, 2, "q", kdt)
        wave(12, ngrp)  # kc 24..31
        project_qt(stq1, 1, 1, 2)  # qts[3]
        attn_end(0)
        attn_end(1)

        # later q-chunk projections ride inside the attention streams, one
        # 512-block (4 DoubleRow pair-MMs, ~1us PE) per insertion, where the
        # 3-group ScalarE lookahead absorbs the PE hiccup
        stq3 = None
        for qb in range(2, NSB):
            attn_begin(qb)
            attn_groups(qb, 0, 3)
            if qb == 2:
                project_qt(stq2, 2, 0, 1)  # qts[4]
            elif qb == 3:
                project_qt(stq2, 2, 1, 2)  # qts[5]
            elif qb == 4:
                project_qt(stq3, 3, 0, 1)  # qts[6]
            elif qb == 5:
                project_qt(stq3, 3, 1, 2)  # qts[7]
            attn_groups(qb, 3, 10)
            if qb == 2:
                stq3 = load_chunk(qT, 3, "q", kdt)
            attn_groups(qb, 10, ngrp)
            attn_end(qb)


def build(s_len=S):
    import concourse.tile as tile
    from concourse import bacc, mybir

    nc = bacc.Bacc(
        "TRN2",
        target_bir_lowering=False,
        debug=False,
        enable_asserts=False,
        num_devices=8,
    )
    f32 = mybir.dt.float32
    bf16 = mybir.dt.bfloat16
    kdt = mybir.dt.float8e4 if FP8_QK else bf16
    if FP8_QK:
        wshape = [P, CD // 2, 2, DK]
        wdt = mybir.dt.float8e4
    else:
        wshape = [P, CD, DK]
        wdt = bf16
    aps = [
        nc.dram_tensor("qT", [NCH, P, CD, CH], kdt, kind="ExternalInput").ap(),
        nc.dram_tensor("kT", [NCH, P, CD, CH], kdt, kind="ExternalInput").ap(),
        nc.dram_tensor("vT", [NCH, P, CD, CH], bf16, kind="ExternalInput").ap(),
        nc.dram_tensor("Wq", wshape, wdt, kind="ExternalInput").ap(),
        nc.dram_tensor("Wk", wshape, wdt, kind="ExternalInput").ap(),
        nc.dram_tensor("Wv", [P, CD, DV], bf16, kind="ExternalInput").ap(),
        nc.dram_tensor("bq", [DK, 1], f32, kind="ExternalInput").ap(),
        nc.dram_tensor("bk", [DK, 1], f32, kind="ExternalInput").ap(),
        nc.dram_tensor("bv", [P, DV], f32, kind="ExternalInput").ap(),
        nc.dram_tensor("out", [S, DV], f32, kind="ExternalOutput").ap(),
    ]
    with tile.TileContext(nc) as tc:
        _emit(tc, aps)
    nc.compile()
    return nc


def make_in_maps(inputs, s_len=S):
    import ml_dtypes

    bf = ml_dtypes.bfloat16
    f8 = ml_dtypes.float8_e4m3
    kdt = f8 if FP8_QK else bf

    def prep_w(w):
        # [d, k] -> [p, c, k] with d = c*128 + p
        w = np.asarray(w, np.float32).reshape(CD, P, -1).transpose(1, 0, 2)
        return np.ascontiguousarray(w).astype(bf)

    def prep_w_pair(w):
        # [d, k] -> [p, c2, i, k] with d = (2*c2 + i)*128 + p
        w = np.asarray(w, np.float32).reshape(CD // 2, 2, P, -1).transpose(2, 0, 1, 3)
        return np.ascontiguousarray(w).astype(f8)

    prep_wqk = prep_w_pair if FP8_QK else prep_w

    weights = {
        "Wq": prep_wqk(inputs["Wq"]),
        "Wk": prep_wqk(inputs["Wk"]),
        "Wv": prep_w(inputs["Wv"]),
        "bq": np.ascontiguousarray(inputs["bq"], dtype=np.float32).reshape(DK, 1),
        "bk": np.ascontiguousarray(inputs["bk"], dtype=np.float32).reshape(DK, 1),
        "bv": np.ascontiguousarray(
            np.broadcast_to(
                np.asarray(inputs["bv"], np.float32).reshape(1, DV), (P, DV)
            )
        ),
    }

    def prep_x(x, dt):
        # [s, d] f32 -> [ci, p, c, s_local] chunk-contiguous staging
        x = np.asarray(x, np.float32).reshape(NCH, CH, CD, P).transpose(0, 3, 2, 1)
        return x.astype(dt)

    in_maps = []
    for i in range(B):
        m = dict(weights)
        m["qT"] = prep_x(inputs["query"][i], kdt)
        m["kT"] = prep_x(inputs["key"][i], kdt)
        m["vT"] = prep_x(inputs["value"][i], bf)
        in_maps.append(m)
    return in_maps


def kernel(**inputs):
    from concourse.bass_utils import run_bass_kernel_spmd

    if "nc" not in _cache:
        _cache["nc"] = build(S)
    nc = _cache["nc"]
    in_maps = make_in_maps(inputs, S)
    res = run_bass_kernel_spmd(nc, in_maps, core_ids=list(range(B)))
    return np.stack([r["out"] for r in res.results], axis=0)


# revision 13
# speedup vs baseline: 1.1467x; 1.1467x over previous
"""Single-head attention (B=8, S=4096, D=1024, DK=DV=128) on 8 TRN2 NeuronCores.

Sharding: data-parallel over batch - one batch element per core, the three
Linear weights replicated. No collectives.

v3 (from the v2 ~212-221us baseline). Trace-driven changes:

1. ScalarE is the steady-state bottleneck (ACTIVATE = (N+352)/1.2 ns each,
   ~16.1us/q-block effective). v2 also put chunk-load DMA triggers on the
   scalar queue (~700ns each inside the ACT FIFO) - v3 issues all x-chunk
   loads on the sync HWDGE ring only (one ring's DMA already fans out to
   all 16 SDMA engines; quarters are sequenced on the same ring so
   projection subtile deps fire as early as before). Weights/biases go on
   gpsimd SWDGE.

2. v2 emitted project_qt(chunk 1..3) between attn_end(qb) and
   attn_begin(qb+1): 16 projection MMs + DVE bias adds sat between the
   q-blocks in the engine FIFOs, starving ScalarE ~3.6us at three q-block
   boundaries. v3 emits those projections inside the next q-block's
   attention stream (after group 2), where PE/DVE have slack.

3. Host staging is chunk-contiguous: x^T is pre-arranged per load-chunk as
   [ci][p][c*CH+s] so each chunk DMA is 128 descriptors x 4-16KB
   contiguous (v2: 256 x 2KB strided).

4. FP8_QK: query/key activations AND Wq/Wk are cast host-side to fp8-e4m3
   (TRN float8e4; values well under the 240 max). The Q/K projections run
   as fp8 DoubleRow matmuls (contraction pairs of d-chunks, 2 MACs/cycle):
   4 pair-MMs per 512-block instead of 8, halving Q/K projection PE time,
   and the Q/K input DMA bytes halve (front is HBM-bound at ~300GB/s/core).
   V path stays bf16 (the output is a near-uniform softmax average, so V
   quantization error does not average out; measured numpy-sim rel_err
   1.1e-2 for this config vs 1.7e-3 baseline, tolerance 2e-2).

Attention machinery unchanged from v2: transposed-score layout, exp on
ScalarE in [128,1024] calls, P^T @ [V | 1] PSUM accumulation with the ones
column producing the softmax denominator, fused DVE normalize.
"""

import math
import os

import numpy as np

B, S, D, DK, DV = 8, 4096, 1024, 128, 128
P = 128
SB = 512  # q-block width (attention) and projection block
CH = 1024  # load chunk (sequence cols per stage load)
CD = D // P  # 8 d-chunks
NSB = S // SB  # 8 q-blocks
NKC = S // P  # 32 key chunks
NCH = S // CH  # 4 load chunks per tensor
JPB = SB // P  # 4 q-subchunks per block
SCALE = 1.0 / math.sqrt(DK)

FP8_QK = os.environ.get("FP8_QK", "1") != "0"  # fp8-e4m3 x_q/x_k/Wq/Wk + DoubleRow projections

_cache = {}

# kept for test.py compat (unused)
XBAR_DUAL = False
SWDGE_QUEUES = 1


def _emit(tc, aps):
    from concourse import mybir

    nc = tc.nc
    bf16 = mybir.dt.bfloat16
    f32 = mybir.dt.float32

    qT, kT, vT, wq, wk, wv, bq, bk, bv, out = aps

    out_ap = out.rearrange("(nb j p) d -> nb p j d", p=P, j=JPB)

    from contextlib import ExitStack

    with ExitStack() as ctx:
        consts = ctx.enter_context(tc.tile_pool(name="consts", bufs=1))
        qkv = ctx.enter_context(tc.tile_pool(name="qkv", bufs=1))
        qtp = ctx.enter_context(tc.tile_pool(name="qt", bufs=NSB))
        stagep = ctx.enter_context(tc.tile_pool(name="stage", bufs=7))
        # deep pt pool: holds one q-block's full stash (16 groups) plus the
        # next one's in-flight tiles
        ptp = ctx.enter_context(tc.tile_pool(name="pt", bufs=22))
        outp = ctx.enter_context(tc.tile_pool(name="outp", bufs=2))
        smallp = ctx.enter_context(tc.tile_pool(name="small", bufs=4))
        psump = ctx.enter_context(tc.tile_pool(name="ps", bufs=2, space="PSUM"))

        # --- constants ---
        if FP8_QK:
            f8 = mybir.dt.float8e4
            wq_sb = consts.tile([P, CD // 2, 2, DK], f8)
            wk_sb = consts.tile([P, CD // 2, 2, DK], f8)
        else:
            wq_sb = consts.tile([P, CD, DK], bf16)
            wk_sb = consts.tile([P, CD, DK], bf16)
        wv_sb = consts.tile([P, CD, DV], bf16)
        # wk first on sync (needed by the first projections), rest on gpsimd
        nc.sync.dma_start(out=wk_sb, in_=wk)
        nc.gpsimd.dma_start(out=wv_sb, in_=wv)
        nc.gpsimd.dma_start(out=wq_sb, in_=wq)
        bq_sb = consts.tile([P, 1], f32)
        bk_sb = consts.tile([P, 1], f32)
        bv_sb = consts.tile([P, DV], f32)
        nc.gpsimd.dma_start(out=bq_sb, in_=bq)
        nc.gpsimd.dma_start(out=bk_sb, in_=bk)
        nc.gpsimd.dma_start(out=bv_sb, in_=bv)

        # warm the exp table set while loads stream
        warm_in = consts.tile([P, 8], f32)
        warm_out = consts.tile([P, 8], f32)
        nc.vector.memset(warm_in, 0.0)
        nc.scalar.activation(warm_out, warm_in, mybir.ActivationFunctionType.Exp)

        # persistent per-core tensors
        kt_sb = qkv.tile([P, S], bf16)  # K^T  [dk, s]
        vp_sb = qkv.tile([P, NKC, DV + 1], bf16)  # V' natural [s%128, chunk, dv+1]
        nc.vector.memset(vp_sb[:, :, DV : DV + 1], 1.0)
        qts = [qtp.tile([P, SB], bf16, tag="qt", name=f"qt{i}") for i in range(NSB)]

        def load_chunk(src_ap, ci, nm, dt, quarters=False):
            # sequential c-splits on the sync ring: full BW (one DMA fans out
            # to all 16 SDMA engines), and the projections' subtile deps on
            # the c dim fire as each split lands.
            st = stagep.tile([P, CD, CH], dt, tag="stage", name=f"st_{nm}{ci}")
            sl = src_ap[ci]
            step = CD // 4 if quarters else CD // 2
            for c0 in range(0, CD, step):
                nc.sync.dma_start(out=st[:, c0 : c0 + step, :], in_=sl[:, c0 : c0 + step, :])
            return st

        def project_x(st, ci, w_sb, b_sb, dst, h_lo, h_hi, nm):
            # dst: callable sb -> (out_ap) receiving the biased block
            for h in range(h_lo, h_hi):
                sb = ci * (CH // SB) + h
                ps = psump.tile([P, SB], f32, tag="sps", bufs=3, name=f"{nm}ps{sb}")
                if FP8_QK:
                    for c2 in range(CD // 2):
                        nc.tensor.matmul(
                            ps,
                            w_sb[:, c2, :, :],
                            st[:, 2 * c2 : 2 * c2 + 2, h * SB : (h + 1) * SB],
                            start=(c2 == 0),
                            stop=(c2 == CD // 2 - 1),
                            perf_mode=mybir.MatmulPerfMode.DoubleRow,
                        )
                else:
                    for c in range(CD):
                        nc.tensor.matmul(
                            ps,
                            w_sb[:, c, :],
                            st[:, c, h * SB : (h + 1) * SB],
                            start=(c == 0),
                            stop=(c == CD - 1),
                        )
                nc.vector.tensor_scalar_add(dst(sb), ps, b_sb)

        def project_kt(st, ci, h_lo=0, h_hi=CH // SB):
            project_x(
                st, ci, wk_sb, bk_sb,
                lambda sb: kt_sb[:, sb * SB : (sb + 1) * SB], h_lo, h_hi, "k",
            )

        def project_qt(st, ci, h_lo=0, h_hi=CH // SB):
            project_x(st, ci, wq_sb, bq_sb, lambda sb: qts[sb], h_lo, h_hi, "q")

        def project_v(st, ci, j_lo=0, j_hi=CH // P):
            # 4 j-outputs packed per [128, 512] psum tile; start=True only on
            # the first matmul per bank (whole-bank has_written clear), later
            # j's first write overwrites on cleared bits.
            for j4 in range(j_lo, j_hi, 4):
                vps = psump.tile(
                    [P, SB], f32, tag="sps", bufs=3, name=f"vps{ci}_{j4}"
                )
                for j in range(j4, j4 + 4):
                    jj = j - j4
                    for c in range(CD):
                        nc.tensor.matmul(
                            vps[:, jj * DV : (jj + 1) * DV],
                            st[:, c, j * P : (j + 1) * P],
                            wv_sb[:, c, :],
                            start=(jj == 0 and c == 0),
                            stop=(c == CD - 1),
                        )
                kk0 = ci * (CH // P) + j4
                nc.vector.tensor_copy(
                    vp_sb[:, kk0 : kk0 + 4, 0:DV],
                    vps.rearrange("p (j d) -> p j d", j=4),
                )

        # --- attention emission helpers ---
        # key chunks grouped 2 per exp call ([128, 1024] ACTIVATEs); the
        # smaller group buys a 3-deep sps rotation (3x2=6 banks) that
        # decouples the scores matmuls from exp WAR jitter.
        #
        # Deferred-PV pipeline: attn_scores stashes each group's exp'd
        # probabilities (pt) in a deep SBUF pool; attn_pv consumes them
        # later - each q-block's PV matmuls ride the NEXT q-block's
        # scores/exp stream.  The ScalarE ACT chain then depends only on
        # scores matmuls (never on PV/ops/DVE), and two q-blocks' worth of
        # exp fits inside the DMA-bound load front with only 2 ops banks.
        groups = [(g * 2, 2) for g in range(NKC // 2)]
        ngrp = len(groups)
        qb_ops = {}
        pt_stash = {}

        def attn_begin(qb):
            opsA = psump.tile([P, 2, DV + 1], f32, tag="ops", bufs=2, name=f"opsA{qb}")
            opsB = psump.tile([P, 2, DV + 1], f32, tag="ops", bufs=2, name=f"opsB{qb}")
            qb_ops[qb] = [opsA[:, 0, :], opsA[:, 1, :], opsB[:, 0, :], opsB[:, 1, :]]

        def attn_scores(qb, g_lo, g_hi):
            for gi in range(g_lo, g_hi):
                k0, gn = groups[gi]
                sps = psump.tile(
                    [P, gn * SB], f32, tag="sps", bufs=3, name=f"sps{qb}_{gi}"
                )
                for h in range(gn):
                    kk = k0 + h
                    nc.tensor.matmul(
                        sps[:, h * SB : (h + 1) * SB],
                        kt_sb[:, kk * P : (kk + 1) * P],
                        qts[qb],
                        start=True,
                        stop=True,
                    )
                pt = ptp.tile([P, gn * SB], bf16, tag="pt", name=f"pt{qb}_{gi}")
                nc.scalar.activation(
                    pt, sps, mybir.ActivationFunctionType.Exp, scale=SCALE
                )
                pt_stash[(qb, gi)] = pt

        def attn_pv(qb, g_lo, g_hi):
            ops = qb_ops[qb]
            for gi in range(g_lo, g_hi):
                k0, gn = groups[gi]
                pt = pt_stash.pop((qb, gi))
                for h in range(gn):
                    kk = k0 + h
                    for j in range(JPB):
                        # start=True clears has_written for the WHOLE bank, so
                        # only the first matmul per bank (j=0 / j=2) may set it;
                        # the partner tile's first write lands on cleared bits
                        # and overwrites (per-element has_written semantics).
                        nc.tensor.matmul(
                            ops[j],
                            pt[:, h * SB + j * P : h * SB + (j + 1) * P],
                            vp_sb[:, kk, :],
                            start=(gi == 0 and h == 0 and j % 2 == 0),
                            stop=(gi == ngrp - 1 and h == gn - 1),
                        )

        def attn_end(qb):
            ops = qb_ops.pop(qb)
            ostage = outp.tile([P, JPB, DV], f32, tag="ostage", name=f"ostage{qb}")
            for j in range(JPB):
                recip = smallp.tile([P, 1], f32, tag="recip", name=f"recip{qb}_{j}")
                nc.vector.reciprocal(recip, ops[j][:, DV : DV + 1])
                nc.vector.scalar_tensor_tensor(
                    ostage[:, j, :],
                    ops[j][:, 0:DV],
                    recip,
                    bv_sb,
                    mybir.AluOpType.mult,
                    mybir.AluOpType.add,
                )
                if j % 2 == 1:  # store halves as they complete (shorter tail)
                    nc.sync.dma_start(
                        out=out_ap[qb][:, j - 1 : j + 1, :],
                        in_=ostage[:, j - 1 : j + 1, :],
                    )

        LAG = 2  # groups the deferred PV trails behind the scores stream

        def front(g_lo, g_hi):
            # load front: qb0's AND qb1's scores+exp ride the DMA-bound
            # window (32 ACTs instead of 16 before the steady state begins);
            # qb0's PV trails LAG groups behind so it never heads the PE FIFO
            # while waiting on exp/DVE.
            for g in range(g_lo, g_hi):
                attn_scores(0, g, g + 1)
                attn_scores(1, g, g + 1)
                if g - LAG >= 0:
                    attn_pv(0, g - LAG, g - LAG + 1)

        # --- software pipeline in emission order (engines run their streams
        # FIFO, so emission order IS the per-engine execution order) ---
        kdt = mybir.dt.float8e4 if FP8_QK else bf16
        stk0 = load_chunk(kT, 0, "k", kdt, quarters=True)
        stq0 = load_chunk(qT, 0, "q", kdt, quarters=True)
        stv0 = load_chunk(vT, 0, "v", bf16, quarters=True)
        # first chunk's projections at block granularity so the first
        # attention groups fire as soon as kc 0..3 are projected
        project_kt(stk0, 0, 0, 1)  # kc 0..3
        project_qt(stq0, 0)  # qt[0], qt[1]
        project_v(stv0, 0, 0, 4)  # vp 0..3

        attn_begin(0)
        front(0, 2)  # kc 0..3
        project_kt(stk0, 0, 1, 2)  # kc 4..7
        project_v(stv0, 0, 4, 8)  # vp 4..7
        stk = load_chunk(kT, 1, "k", kdt)
        stv = load_chunk(vT, 1, "v", bf16)
        front(2, 4)  # kc 4..7
        project_kt(stk, 1)
        project_v(stv, 1)
        stk = load_chunk(kT, 2, "k", kdt)
        stv = load_chunk(vT, 2, "v", bf16)
        front(4, 8)  # kc 8..15, needs chunk 1
        project_kt(stk, 2)
        project_v(stv, 2)
        stq1 = load_chunk(qT, 1, "q", kdt)
        stk = load_chunk(kT, 3, "k", kdt)
        stv = load_chunk(vT, 3, "v", bf16)
        front(8, 12)  # kc 16..23, needs chunk 2
        project_kt(stk, 3)
        project_v(stv, 3)
        project_qt(stq1, 1, 0, 1)  # qts[2]
        stq2 = load_chunk(qT, 2, "q", kdt)
        front(12, ngrp)  # kc 24..31
        attn_pv(0, ngrp - LAG, ngrp)
        attn_end(0)

        # steady state: slot i runs q-block i's deferred PV alongside
        # q-block (i+1)'s scores/exp; one qt-projection 512-block (4
        # DoubleRow pair-MMs, ~1us PE) is inserted per slot where needed.
        stq3 = None
        for i in range(1, NSB):
            attn_begin(i)
            nxt = i + 1
            for g in range(ngrp):
                if nxt < NSB:
                    attn_scores(nxt, g, g + 1)
                if g - LAG >= 0:
                    attn_pv(i, g - LAG, g - LAG + 1)
                if g == 3:
                    if i == 1:
                        project_qt(stq1, 1, 1, 2)  # qts[3]
                    elif i == 2:
                        project_qt(stq2, 2, 0, 1)  # qts[4]
                    elif i == 3:
                        project_qt(stq2, 2, 1, 2)  # qts[5]
                    elif i == 4:
                        project_qt(stq3, 3, 0, 1)  # qts[6]
                    elif i == 5:
                        project_qt(stq3, 3, 1, 2)  # qts[7]
                if g == 8 and i == 1:
                    stq3 = load_chunk(qT, 3, "q", kdt)
            attn_pv(i, ngrp - LAG, ngrp)
            attn_end(i)


def build(s_len=S):
    import concourse.tile as tile
    from concourse import bacc, mybir

    nc = bacc.Bacc(
        "TRN2",
        target_bir_lowering=False,
        debug=False,
        enable_asserts=False,
        num_devices=8,
    )
    f32 = mybir.dt.float32
    bf16 = mybir.dt.bfloat16
    kdt = mybir.dt.float8e4 if FP8_QK else bf16
    if FP8_QK:
        wshape = [P, CD // 2, 2, DK]
        wdt = mybir.dt.float8e4
    else:
        wshape = [P, CD, DK]
        wdt = bf16
    aps = [
        nc.dram_tensor("qT", [NCH, P, CD, CH], kdt, kind="ExternalInput").ap(),
        nc.dram_tensor("kT", [NCH, P, CD, CH], kdt, kind="ExternalInput").ap(),
        nc.dram_tensor("vT", [NCH, P, CD, CH], bf16, kind="ExternalInput").ap(),
        nc.dram_tensor("Wq", wshape, wdt, kind="ExternalInput").ap(),
        nc.dram_tensor("Wk", wshape, wdt, kind="ExternalInput").ap(),
        nc.dram_tensor("Wv", [P, CD, DV], bf16, kind="ExternalInput").ap(),
        nc.dram_tensor("bq", [DK, 1], f32, kind="ExternalInput").ap(),
        nc.dram_tensor("bk", [DK, 1], f32, kind="ExternalInput").ap(),
        nc.dram_tensor("bv", [P, DV], f32, kind="ExternalInput").ap(),
        nc.dram_tensor("out", [S, DV], f32, kind="ExternalOutput").ap(),
    ]
    with tile.TileContext(nc) as tc:
        _emit(tc, aps)
    nc.compile()
    return nc


def make_in_maps(inputs, s_len=S):
    import ml_dtypes

    bf = ml_dtypes.bfloat16
    f8 = ml_dtypes.float8_e4m3
    kdt = f8 if FP8_QK else bf

    def prep_w(w):
        # [d, k] -> [p, c, k] with d = c*128 + p
        w = np.asarray(w, np.float32).reshape(CD, P, -1).transpose(1, 0, 2)
        return np.ascontiguousarray(w).astype(bf)

    def prep_w_pair(w):
        # [d, k] -> [p, c2, i, k] with d = (2*c2 + i)*128 + p
        w = np.asarray(w, np.float32).reshape(CD // 2, 2, P, -1).transpose(2, 0, 1, 3)
        return np.ascontiguousarray(w).astype(f8)

    prep_wqk = prep_w_pair if FP8_QK else prep_w

    weights = {
        "Wq": prep_wqk(inputs["Wq"]),
        "Wk": prep_wqk(inputs["Wk"]),
        "Wv": prep_w(inputs["Wv"]),
        "bq": np.ascontiguousarray(inputs["bq"], dtype=np.float32).reshape(DK, 1),
        "bk": np.ascontiguousarray(inputs["bk"], dtype=np.float32).reshape(DK, 1),
        "bv": np.ascontiguousarray(
            np.broadcast_to(
                np.asarray(inputs["bv"], np.float32).reshape(1, DV), (P, DV)
            )
        ),
    }

    def prep_x(x, dt):
        # [s, d] f32 -> [ci, p, c, s_local] chunk-contiguous staging
        x = np.asarray(x, np.float32).reshape(NCH, CH, CD, P).transpose(0, 3, 2, 1)
        return x.astype(dt)

    in_maps = []
    for i in range(B):
        m = dict(weights)
        m["qT"] = prep_x(inputs["query"][i], kdt)
        m["kT"] = prep_x(inputs["key"][i], kdt)
        m["vT"] = prep_x(inputs["value"][i], bf)
        in_maps.append(m)
    return in_maps


def kernel(**inputs):
    from concourse.bass_utils import run_bass_kernel_spmd

    if "nc" not in _cache:
        _cache["nc"] = build(S)
    nc = _cache["nc"]
    in_maps = make_in_maps(inputs, S)
    res = run_bass_kernel_spmd(nc, in_maps, core_ids=list(range(B)))
    return np.stack([r["out"] for r in res.results], axis=0)
